# revision 30
# baseline (speedup 1.0000x reference)
"""Multi-head attention (B=2, S=2048, D=1024, H=16, causal) on 8 TRN2 NeuronCores.

Sharding: core c -> (batch b = c//4, head-group hg = c%4). Each core:
  - projects its batch's query/key/value against a 256-row slice of Wq/Wk/Wv
    (4 heads of 64 dims),
  - runs causal attention for those 4 heads (scores computed transposed,
    exp on ACT with fused 1/8 scale, row-sums via a ones-column in V),
  - multiplies by the matching 256-column slice of Wo -> partial [2048, 1024].
Host sums the 4 partials per batch (the tensor-parallel all-reduce) and stacks.

Performance structure: the TRN2 PE ramps to 2.4 GHz only after ~3us of
continuous busy time and drops to 1.2 GHz after any idle gap, and the per-chunk
exp stream on ACT is slower than the score+PV matmuls at full clock. So all
projection / output-projection matmul work is emitted as "filler" units
interleaved into the attention stream: whenever attention would wait on ACT,
the PE has independent proj/oproj work queued behind it. Everything runs in
bf16 (full-rate matmuls at any width, half the DMA/LDWEIGHTS traffic);
accumulation stays fp32 in PSUM. Normalization uses a fast DVE reciprocal and
a pair-packed selector matmul to broadcast the per-token scales.
"""

import sys

for _p in ("/opt/trn_rl_repo", "/root/.axon_site/_ro/trn_rl_repo"):
    if _p not in sys.path:
        sys.path.append(_p)

from collections import deque

import numpy as np
import ml_dtypes

import concourse.bacc as bacc
import concourse.tile as tile
import concourse.mybir as mybir
from concourse.bass import MemorySpace
from concourse.bass_utils import run_bass_kernel_spmd

f32 = mybir.dt.float32
f32r = mybir.dt.float32r
bf16 = mybir.dt.bfloat16
Exp = mybir.ActivationFunctionType.Exp

B, S, D, H = 2, 2048, 1024, 16
HD = 64            # head dim
NH = 4             # heads per core
DO = NH * HD       # 256 projection out-dims per core
NCORES = 8
KI = D // 128      # 8 contraction chunks for the projections
QT = 512           # query tile
NQT = S // QT      # 4
KT = 128           # key chunk (contraction tile for PV)
NKT = S // KT      # 16
NT = QT // KT      # 4 key chunks per query block

_cache: dict = {}

_opts = {"lead": 5, "ratio": 0.5, "reserve": 8}


def _build():
    nc = bacc.Bacc("TRN2", target_bir_lowering=False, debug=False,
                   num_devices=NCORES)

    xqT_d = nc.dram_tensor("xqT", [D, S], bf16, kind="ExternalInput").ap()
    xkT_d = nc.dram_tensor("xkT", [D, S], bf16, kind="ExternalInput").ap()
    xvT_d = nc.dram_tensor("xvT", [D, S], bf16, kind="ExternalInput").ap()
    wqT_d = nc.dram_tensor("wqT", [D, DO], bf16, kind="ExternalInput").ap()
    wkT_d = nc.dram_tensor("wkT", [D, DO], bf16, kind="ExternalInput").ap()
    wvT_d = nc.dram_tensor("wvT", [D, DO], bf16, kind="ExternalInput").ap()
    woT_d = nc.dram_tensor("woT", [DO, D], bf16, kind="ExternalInput").ap()
    cmask_d = nc.dram_tensor("cmask", [128, KT], bf16, kind="ExternalInput").ap()
    selr_d = nc.dram_tensor("selr", [128, 2 * HD * 2], f32r,
                            kind="ExternalInput").ap()
    out_d = nc.dram_tensor("out", [S, D], f32, kind="ExternalOutput").ap()

    with tile.TileContext(nc) as tc:
        with (
            tc.tile_pool(name="wpool", bufs=1) as wpool,
            tc.tile_pool(name="cpool", bufs=1) as cpool,
            tc.tile_pool(name="persist", bufs=1) as persist,
            tc.tile_pool(name="xin", bufs=12) as xin,
            tc.tile_pool(name="ptp", bufs=8) as ptp,
            tc.tile_pool(name="small", bufs=2) as small,
            tc.tile_pool(name="obuf", bufs=2) as obuf,
            tc.tile_pool(name="psS", bufs=3, space=MemorySpace.PSUM) as psS,
            tc.tile_pool(name="psA", bufs=2, space=MemorySpace.PSUM) as psA,
            tc.tile_pool(name="psO", bufs=2, space=MemorySpace.PSUM) as psO,
            tc.tile_pool(name="psN", bufs=1, space=MemorySpace.PSUM) as psN,
        ):
            _emit(nc, wpool, cpool, persist, xin, ptp, small, obuf,
                  psS, psA, psO, psN, xqT_d, xkT_d, xvT_d, wqT_d, wkT_d,
                  wvT_d, woT_d, cmask_d, selr_d, out_d)

    nc.compile()
    return nc


def _emit(nc, wpool, cpool, persist, xin, ptp, small, obuf, psS, psA, psO, psN,
          xqT_d, xkT_d, xvT_d, wqT_d, wkT_d, wvT_d, woT_d, cmask_d, selr_d,
          out_d):
    # ---- weights + constants; x DMAs issued up front (12 tiles coexist).
    # DMA order is the arrival order: tri (tiny, needed at first mask), then
    # weights/x interleaved so each proj unit's input lands just before the
    # PE reaches it; wo is only needed by oproj (~100us in) so it goes last.
    tri_sb = cpool.tile([128, KT], bf16, tag="tri")
    nc.sync.dma_start(tri_sb[:], cmask_d)
    xq_t, xk_t, xv_t = [], [], []

    def dma_x(lst, dram, t, name):
        ts = slice(t * QT, (t + 1) * QT)
        tl = xin.tile([128, KI, QT], bf16, tag="xin", name=f"{name}{t}")
        nc.sync.dma_start(tl[:], dram[:, ts].rearrange("(k p) n -> p k n",
                                                       p=128))
        lst.append(tl)

    wq_sb = wpool.tile([128, KI, DO], bf16, tag="wq")
    nc.sync.dma_start(wq_sb[:], wqT_d.rearrange("(k p) n -> p k n", p=128))
    dma_x(xq_t, xqT_d, 0, "xq")
    wk_sb = wpool.tile([128, KI, DO], bf16, tag="wk")
    nc.sync.dma_start(wk_sb[:], wkT_d.rearrange("(k p) n -> p k n", p=128))
    dma_x(xk_t, xkT_d, 0, "xk")
    dma_x(xq_t, xqT_d, 1, "xq")
    wv_sb = wpool.tile([128, KI, DO], bf16, tag="wv")
    nc.sync.dma_start(wv_sb[:], wvT_d.rearrange("(k p) n -> p k n", p=128))
    dma_x(xv_t, xvT_d, 0, "xv")
    dma_x(xk_t, xkT_d, 1, "xk")
    dma_x(xv_t, xvT_d, 1, "xv")
    wo_sb = wpool.tile([128, DO // 128, D], bf16, tag="wo")
    nc.sync.dma_start(wo_sb[:], woT_d.rearrange("(k p) n -> p k n", p=128))
    for t in range(2, NQT):
        dma_x(xq_t, xqT_d, t, "xq")
        dma_x(xk_t, xkT_d, t, "xk")
        dma_x(xv_t, xvT_d, t, "xv")

    # selector for the pair-packed reciprocal broadcast (host-built since
    # engine writes must start at 32-aligned partitions):
    # sel[p, c] = 1 iff p == 32 * (c // HD); rowsums are parked at
    # partitions {0,32,64,96} of the rs tile.
    sel = cpool.tile([128, 2 * HD * 2], f32r, tag="sel")
    nc.sync.dma_start(sel[:], selr_d)
    vones_f = cpool.tile([128, NT * NH], f32, tag="vones_f")
    nc.gpsimd.memset(vones_f[:], 1.0)
    vones = cpool.tile([128, NT * NH], bf16, tag="vones")
    nc.vector.tensor_copy(vones[:], vones_f[:])

    # ---- per-block persistent intermediates ----
    # qT/kT/oT blocks: [256, QT] as [128 parts, 2 chunks, QT]
    #   head j lives in chunk j//2, partitions (j%2)*64 ..+64
    qTt = [persist.tile([128, 2, QT], bf16, tag=f"qT{t}", name=f"qT{t}")
           for t in range(NQT)]
    kTt = [persist.tile([128, 2, QT], bf16, tag=f"kT{t}", name=f"kT{t}")
           for t in range(NQT)]
    oTt = [persist.tile([128, 2, QT], bf16, tag=f"oT{t}", name=f"oT{t}")
           for t in range(NQT)]
    # v blocks, natural layout + ones column: [tokk part, ktc, head, 65]
    vt = [persist.tile([128, NT, NH, HD + 1], bf16, tag=f"v{t}", name=f"v{t}")
          for t in range(NQT)]

    # ---- filler units: (cycles, tag, closure) drained into the PE stream ----
    fillers = deque()
    state = {"deficit": 0.0}
    RATIO = _opts["ratio"]

    def drain(cycles, reserve=0):
        state["deficit"] += cycles * RATIO
        while (len(fillers) > reserve
               and state["deficit"] >= fillers[0][0]):
            cyc, _tag, fn = fillers.popleft()
            fn()
            state["deficit"] -= cyc

    def force_units(pred):
        while fillers and pred(fillers[0][1]):
            _cyc, _tag, fn = fillers.popleft()
            fn()

    # Only DVE (and ACT) can drain PSUM on TRN2; ACT is saturated by the exp
    # stream, so every PSUM->SBUF copy goes to DVE and the SBUF-only work
    # (masks, rowsum parking) goes to Pool.
    cp_eng = [nc.vector, nc.vector]

    def proj_units(t):
        units = []
        for d in range(2):
            def qunit(t=t, d=d):
                ps = psA.tile([128, QT], f32, tag="ps", name="psq")
                for ki in range(KI):
                    nc.tensor.matmul(
                        ps[:], wq_sb[:, ki, d * 128:(d + 1) * 128],
                        xq_t[t][:, ki, :], start=(ki == 0), stop=(ki == KI - 1))
                cp_eng[d].tensor_copy(qTt[t][:, d, :], ps[:])
            units.append((8 * QT, ("proj", t), qunit))
        for d in range(2):
            def kunit(t=t, d=d):
                ps = psA.tile([128, QT], f32, tag="ps", name="psk")
                for ki in range(KI):
                    nc.tensor.matmul(
                        ps[:], wk_sb[:, ki, d * 128:(d + 1) * 128],
                        xk_t[t][:, ki, :], start=(ki == 0), stop=(ki == KI - 1))
                cp_eng[d].tensor_copy(kTt[t][:, d, :], ps[:])
            units.append((8 * QT, ("proj", t), kunit))
        for tt in range(NT):
            def vunit(t=t, tt=tt):
                if tt == 0:
                    nc.gpsimd.tensor_copy(
                        vt[t][:, :, :, HD],
                        vones[:].rearrange("p (a b) -> p a b", a=NT))
                psv = psA.tile([128, DO], f32, tag="ps", name="psv")
                for ki in range(KI):
                    nc.tensor.matmul(
                        psv[:], xv_t[t][:, ki, tt * KT:(tt + 1) * KT],
                        wv_sb[:, ki, :], start=(ki == 0), stop=(ki == KI - 1))
                cp_eng[tt % 2].tensor_copy(
                    vt[t][:, tt, :, 0:HD],
                    psv[:].rearrange("p (h e) -> p h e", h=NH))
            units.append((8 * DO, ("proj", t), vunit))
        return units

    def oproj_units(t):
        units = []
        for mtt in range(NT):
            for n in range(D // QT):
                def ounit(t=t, mtt=mtt, n=n):
                    mt = t * NT + mtt
                    ps = psA.tile([128, QT], f32, tag="ps", name="pso2")
                    for kc in range(DO // 128):
                        nc.tensor.matmul(
                            ps[:], oTt[t][:, kc, mtt * KT:(mtt + 1) * KT],
                            wo_sb[:, kc, n * QT:(n + 1) * QT],
                            start=(kc == 0), stop=(kc == DO // 128 - 1))
                    ob = obuf.tile([128, QT], f32, tag="ob", name="ob")
                    cp_eng[(mtt + n) % 2].tensor_copy(ob[:], ps[:])
                    nc.sync.dma_start(
                        out_d[mt * 128:(mt + 1) * 128, n * QT:(n + 1) * QT],
                        ob[:])
                units.append((2 * QT, ("oproj", t), ounit))
        return units

    # ---- attention: scores -> exp (ACT) -> mask (Pool) -> PV, with the
    # filler stream keeping the PE dense; normalization is deferred into the
    # next block's stream so its serial chain hides behind attention work ----
    def attn_block(qt, reserve=0):
        LEAD = _opts["lead"]
        nkt = (qt + 1) * NT
        ouns = []
        for j in range(NH):
            poff = (j % 2) * HD
            d = j // 2
            qh = qTt[qt][poff:poff + HD, d, :]
            pso = psO.tile([HD + 1, QT], f32, tag="pso", name="pso")
            window = {}
            for step in range(nkt + LEAD):
                # PV first: its pt has been ready for LEAD chunks, so the PE
                # always has runnable work queued ahead of the (possibly
                # ACT-paced) score matmul behind it.
                if step >= LEAD:
                    kt = step - LEAD
                    co, w, pt = window.pop(kt)
                    nc.tensor.matmul(
                        pso[:, co:QT], vt[kt // NT][:, kt % NT, j, :],
                        pt[:, 0:w], start=(kt == 0), stop=(kt == nkt - 1))
                    drain(w, reserve)
                if step < nkt:
                    kt = step
                    r = kt - qt * NT
                    co = max(r, 0) * KT
                    w = QT - co
                    kh = kTt[kt // NT][poff:poff + HD, d,
                                       (kt % NT) * KT:(kt % NT + 1) * KT]
                    pss = psS.tile([128, QT], f32, tag="pss", name="pss")
                    nc.tensor.matmul(pss[:, 0:w], kh, qh[:, co:QT],
                                     start=True, stop=True)
                    pt = ptp.tile([128, QT], bf16, tag="pt", name="pt")
                    nc.scalar.activation(pt[:, 0:w], pss[:, 0:w], Exp,
                                         scale=0.125)
                    if r >= 0:
                        nc.gpsimd.tensor_mul(pt[:, 0:KT], pt[:, 0:KT],
                                             tri_sb[:])
                    window[kt] = (co, w, pt)
                    drain(w, reserve)
            # free the PSUM accumulator (DVE: keeps ACT's exp stream dense).
            # bufs=9: norm(qt) is emitted after attn(qt+1), so two blocks'
            # worth of oun tiles (8) are alive at once.
            oun = small.tile([HD + 1, QT], f32, tag="oun", name="oun", bufs=9)
            nc.vector.tensor_copy(oun[:], pso[:])
            if j == 0:
                rs = small.tile([128, QT], f32, tag="rs", name="rs")
                nc.gpsimd.memset(rs[:], 1.0)
            # park the rowsum at partition 32*j as soon as the head finishes
            # (DVE, directly behind the oun copy: Pool here would turn the
            # next block's in-order DVE queue into a cross-engine barrier)
            nc.vector.tensor_copy(rs[32 * j:32 * j + 1, :],
                                  oun[HD:HD + 1, :])
            ouns.append(oun)
        # rowsums -> 1/x on DVE (fast approx; sums are in [1, ~12000]).
        # Unused partitions hold 1.0 so the reciprocal stays finite; the
        # selector matmul zeroes them out anyway.
        rec_f = small.tile([128, QT], f32, tag="rec", name="rec")
        nc.vector.reciprocal_approx_fast(rec_f[:], rs[:])
        rec_r = small.tile([128, QT], f32r, tag="recr", name="recr")
        with nc.allow_low_precision(reason="f32r normalization scale"):
            nc.vector.tensor_copy(rec_r[:], rec_f[:])

        def norm(qt=qt, ouns=ouns, rec_r=rec_r):
            for dd in range(2):
                psb = psN.tile([128, QT], f32, tag="psb", name="psb")
                nc.tensor.matmul(psb[:], sel[:, dd * 128:(dd + 1) * 128],
                                 rec_r[:], start=True, stop=True)
                for pp in range(2):
                    nc.vector.tensor_mul(
                        oTt[qt][pp * HD:(pp + 1) * HD, dd, :],
                        ouns[2 * dd + pp][0:HD, :], psb[pp * HD:(pp + 1) * HD, :])
                drain(QT, 0)
        return norm

    # ---- schedule ----
    for u in proj_units(0):
        u[2]()
    fillers.extend(proj_units(1))
    fillers.extend(proj_units(2))
    fillers.extend(proj_units(3))

    norm_prev = attn_block(0)

    force_units(lambda tag: tag == ("proj", 1))
    norm_prev2 = attn_block(1)
    norm_prev()                       # norm0: its recip chain ran during attn1
    fillers.extend(oproj_units(0))
    norm_prev = norm_prev2

    force_units(lambda tag: tag == ("proj", 2))
    norm_prev2 = attn_block(2)
    norm_prev()
    fillers.extend(oproj_units(1))
    norm_prev = norm_prev2

    force_units(lambda tag: tag == ("proj", 3))
    norm_prev2 = attn_block(3, reserve=_opts["reserve"])
    norm_prev()
    fillers.extend(oproj_units(2))
    # drain everything except a small reserve to cover norm3's serial chain
    while len(fillers) > _opts["reserve"]:
        _cyc, _tag, fn = fillers.popleft()
        fn()
    norm_prev2()
    fillers.extend(oproj_units(3))
    force_units(lambda tag: True)


def _mask_tiles() -> np.ndarray:
    i = np.arange(128)[:, None]
    j = np.arange(KT)[None, :]
    return (j >= i).astype(ml_dtypes.bfloat16)


def _sel_mat() -> np.ndarray:
    p = np.arange(128)[:, None]
    c = np.arange(2 * HD * 2)[None, :]
    return (p == 32 * (c // HD)).astype(np.float32)


def make_in_maps(query, key, value, Wq, Wk, Wv, Wo):
    bf = ml_dtypes.bfloat16
    query = np.asarray(query, np.float32)
    key = np.asarray(key, np.float32)
    value = np.asarray(value, np.float32)
    Wq = np.asarray(Wq, np.float32)
    Wk = np.asarray(Wk, np.float32)
    Wv = np.asarray(Wv, np.float32)
    Wo = np.asarray(Wo, np.float32)
    cm = _mask_tiles()
    in_maps = []
    for c in range(NCORES):
        b, hg = divmod(c, NCORES // B)
        sl = slice(hg * DO, (hg + 1) * DO)
        in_maps.append({
            "xqT": np.ascontiguousarray(query[b].T).astype(bf),
            "xkT": np.ascontiguousarray(key[b].T).astype(bf),
            "xvT": np.ascontiguousarray(value[b].T).astype(bf),
            "wqT": np.ascontiguousarray(Wq[sl].T).astype(bf),
            "wkT": np.ascontiguousarray(Wk[sl].T).astype(bf),
            "wvT": np.ascontiguousarray(Wv[sl].T).astype(bf),
            "woT": np.ascontiguousarray(Wo[:, sl].T).astype(bf),
            "cmask": cm,
            "selr": _sel_mat(),
        })
    return in_maps


def kernel(query, key, value, freqs_complex_form, mask, Wq, Wk, Wv, Wo):
    if "nc" not in _cache:
        _cache["nc"] = _build()
    nc = _cache["nc"]
    in_maps = make_in_maps(query, key, value, Wq, Wk, Wv, Wo)
    res = run_bass_kernel_spmd(nc, in_maps, list(range(NCORES)))
    parts = [res.results[c]["out"] for c in range(NCORES)]
    npg = NCORES // B
    return np.stack(
        [np.sum(parts[b * npg:(b + 1) * npg], axis=0) for b in range(B)]
    ).astype(np.float32)


# revision 31
# speedup vs baseline: 1.0083x; 1.0083x over previous
"""Multi-head attention (B=2, S=2048, D=1024, H=16, causal) on 8 TRN2 NeuronCores.

Sharding: core c -> (batch b = c//4, head-group hg = c%4). Each core:
  - projects its batch's query/key/value against a 256-row slice of Wq/Wk/Wv
    (4 heads of 64 dims),
  - runs causal attention for those 4 heads (scores computed transposed,
    exp on ACT with fused 1/8 scale, row-sums via a ones-column in V),
  - multiplies by the matching 256-column slice of Wo -> partial [2048, 1024].
Host sums the 4 partials per batch (the tensor-parallel all-reduce) and stacks.

Performance structure: the TRN2 PE ramps to 2.4 GHz only after ~3us of
continuous busy time and drops to 1.2 GHz after any idle gap, and the per-chunk
exp stream on ACT is slower than the score+PV matmuls at full clock. So all
projection / output-projection matmul work is emitted as "filler" units
interleaved into the attention stream: whenever attention would wait on ACT,
the PE has independent proj/oproj work queued behind it. Everything runs in
bf16 (full-rate matmuls at any width, half the DMA/LDWEIGHTS traffic);
accumulation stays fp32 in PSUM. Normalization uses a fast DVE reciprocal and
a pair-packed selector matmul to broadcast the per-token scales.
"""

import sys

for _p in ("/opt/trn_rl_repo", "/root/.axon_site/_ro/trn_rl_repo"):
    if _p not in sys.path:
        sys.path.append(_p)

from collections import deque

import numpy as np
import ml_dtypes

import concourse.bacc as bacc
import concourse.tile as tile
import concourse.mybir as mybir
from concourse.bass import MemorySpace
from concourse.bass_utils import run_bass_kernel_spmd

f32 = mybir.dt.float32
f32r = mybir.dt.float32r
bf16 = mybir.dt.bfloat16
Exp = mybir.ActivationFunctionType.Exp

B, S, D, H = 2, 2048, 1024, 16
HD = 64            # head dim
NH = 4             # heads per core
DO = NH * HD       # 256 projection out-dims per core
NCORES = 8
KI = D // 128      # 8 contraction chunks for the projections
QT = 512           # query tile
NQT = S // QT      # 4
KT = 128           # key chunk (contraction tile for PV)
NKT = S // KT      # 16
NT = QT // KT      # 4 key chunks per query block

_cache: dict = {}

_opts = {"lead": 2, "ratio": 0.5, "reserve": 8}


def _build():
    nc = bacc.Bacc("TRN2", target_bir_lowering=False, debug=False,
                   num_devices=NCORES)

    xqT_d = nc.dram_tensor("xqT", [D, S], bf16, kind="ExternalInput").ap()
    xkT_d = nc.dram_tensor("xkT", [D, S], bf16, kind="ExternalInput").ap()
    xvT_d = nc.dram_tensor("xvT", [D, S], bf16, kind="ExternalInput").ap()
    wqT_d = nc.dram_tensor("wqT", [D, DO], bf16, kind="ExternalInput").ap()
    wkT_d = nc.dram_tensor("wkT", [D, DO], bf16, kind="ExternalInput").ap()
    wvT_d = nc.dram_tensor("wvT", [D, DO], bf16, kind="ExternalInput").ap()
    woT_d = nc.dram_tensor("woT", [DO, D], bf16, kind="ExternalInput").ap()
    cmask_d = nc.dram_tensor("cmask", [128, KT], bf16, kind="ExternalInput").ap()
    selr_d = nc.dram_tensor("selr", [128, 2 * HD * 2], f32r,
                            kind="ExternalInput").ap()
    out_d = nc.dram_tensor("out", [S, D], f32, kind="ExternalOutput").ap()

    with tile.TileContext(nc) as tc:
        with (
            tc.tile_pool(name="wpool", bufs=1) as wpool,
            tc.tile_pool(name="cpool", bufs=1) as cpool,
            tc.tile_pool(name="persist", bufs=1) as persist,
            tc.tile_pool(name="xin", bufs=12) as xin,
            tc.tile_pool(name="ptp", bufs=8) as ptp,
            tc.tile_pool(name="small", bufs=2) as small,
            tc.tile_pool(name="obuf", bufs=2) as obuf,
            tc.tile_pool(name="psS", bufs=3, space=MemorySpace.PSUM) as psS,
            tc.tile_pool(name="psA", bufs=2, space=MemorySpace.PSUM) as psA,
            tc.tile_pool(name="psO", bufs=2, space=MemorySpace.PSUM) as psO,
            tc.tile_pool(name="psN", bufs=1, space=MemorySpace.PSUM) as psN,
        ):
            _emit(nc, wpool, cpool, persist, xin, ptp, small, obuf,
                  psS, psA, psO, psN, xqT_d, xkT_d, xvT_d, wqT_d, wkT_d,
                  wvT_d, woT_d, cmask_d, selr_d, out_d)

    nc.compile()
    return nc


def _emit(nc, wpool, cpool, persist, xin, ptp, small, obuf, psS, psA, psO, psN,
          xqT_d, xkT_d, xvT_d, wqT_d, wkT_d, wvT_d, woT_d, cmask_d, selr_d,
          out_d):
    # ---- weights + constants; x DMAs issued up front (12 tiles coexist).
    # DMA order is the arrival order: tri (tiny, needed at first mask), then
    # weights/x interleaved so each proj unit's input lands just before the
    # PE reaches it; wo is only needed by oproj (~100us in) so it goes last.
    tri_sb = cpool.tile([128, KT], bf16, tag="tri")
    nc.sync.dma_start(tri_sb[:], cmask_d)
    xq_t, xk_t, xv_t = [], [], []

    def dma_x(lst, dram, t, name):
        ts = slice(t * QT, (t + 1) * QT)
        tl = xin.tile([128, KI, QT], bf16, tag="xin", name=f"{name}{t}")
        nc.sync.dma_start(tl[:], dram[:, ts].rearrange("(k p) n -> p k n",
                                                       p=128))
        lst.append(tl)

    wq_sb = wpool.tile([128, KI, DO], bf16, tag="wq")
    nc.sync.dma_start(wq_sb[:], wqT_d.rearrange("(k p) n -> p k n", p=128))
    dma_x(xq_t, xqT_d, 0, "xq")
    wk_sb = wpool.tile([128, KI, DO], bf16, tag="wk")
    nc.sync.dma_start(wk_sb[:], wkT_d.rearrange("(k p) n -> p k n", p=128))
    dma_x(xk_t, xkT_d, 0, "xk")
    dma_x(xq_t, xqT_d, 1, "xq")
    wv_sb = wpool.tile([128, KI, DO], bf16, tag="wv")
    nc.sync.dma_start(wv_sb[:], wvT_d.rearrange("(k p) n -> p k n", p=128))
    dma_x(xv_t, xvT_d, 0, "xv")
    dma_x(xk_t, xkT_d, 1, "xk")
    dma_x(xv_t, xvT_d, 1, "xv")
    wo_sb = wpool.tile([128, DO // 128, D], bf16, tag="wo")
    nc.sync.dma_start(wo_sb[:], woT_d.rearrange("(k p) n -> p k n", p=128))
    for t in range(2, NQT):
        dma_x(xq_t, xqT_d, t, "xq")
        dma_x(xk_t, xkT_d, t, "xk")
        dma_x(xv_t, xvT_d, t, "xv")

    # selector for the pair-packed reciprocal broadcast (host-built since
    # engine writes must start at 32-aligned partitions):
    # sel[p, c] = 1 iff p == 32 * (c // HD); rowsums are parked at
    # partitions {0,32,64,96} of the rs tile.
    sel = cpool.tile([128, 2 * HD * 2], f32r, tag="sel")
    nc.sync.dma_start(sel[:], selr_d)
    vones_f = cpool.tile([128, NT * NH], f32, tag="vones_f")
    nc.gpsimd.memset(vones_f[:], 1.0)
    vones = cpool.tile([128, NT * NH], bf16, tag="vones")
    nc.vector.tensor_copy(vones[:], vones_f[:])

    # ---- per-block persistent intermediates ----
    # qT/kT/oT blocks: [256, QT] as [128 parts, 2 chunks, QT]
    #   head j lives in chunk j//2, partitions (j%2)*64 ..+64
    qTt = [persist.tile([128, 2, QT], bf16, tag=f"qT{t}", name=f"qT{t}")
           for t in range(NQT)]
    kTt = [persist.tile([128, 2, QT], bf16, tag=f"kT{t}", name=f"kT{t}")
           for t in range(NQT)]
    oTt = [persist.tile([128, 2, QT], bf16, tag=f"oT{t}", name=f"oT{t}")
           for t in range(NQT)]
    # v blocks, natural layout + ones column: [tokk part, ktc, head, 65]
    vt = [persist.tile([128, NT, NH, HD + 1], bf16, tag=f"v{t}", name=f"v{t}")
          for t in range(NQT)]

    # ---- filler units: (cycles, tag, closure) drained into the PE stream ----
    fillers = deque()
    state = {"deficit": 0.0}
    RATIO = _opts["ratio"]

    def drain(cycles, reserve=0):
        state["deficit"] += cycles * RATIO
        while (len(fillers) > reserve
               and state["deficit"] >= fillers[0][0]):
            cyc, _tag, fn = fillers.popleft()
            fn()
            state["deficit"] -= cyc

    def force_units(pred):
        while fillers and pred(fillers[0][1]):
            _cyc, _tag, fn = fillers.popleft()
            fn()

    # Only DVE (and ACT) can drain PSUM on TRN2; ACT is saturated by the exp
    # stream, so every PSUM->SBUF copy goes to DVE and the SBUF-only work
    # (masks, rowsum parking) goes to Pool.
    cp_eng = [nc.vector, nc.vector]

    def proj_units(t):
        units = []
        for d in range(2):
            def qunit(t=t, d=d):
                ps = psA.tile([128, QT], f32, tag="ps", name="psq")
                for ki in range(KI):
                    nc.tensor.matmul(
                        ps[:], wq_sb[:, ki, d * 128:(d + 1) * 128],
                        xq_t[t][:, ki, :], start=(ki == 0), stop=(ki == KI - 1))
                cp_eng[d].tensor_copy(qTt[t][:, d, :], ps[:])
            units.append((8 * QT, ("proj", t), qunit))
        for d in range(2):
            def kunit(t=t, d=d):
                ps = psA.tile([128, QT], f32, tag="ps", name="psk")
                for ki in range(KI):
                    nc.tensor.matmul(
                        ps[:], wk_sb[:, ki, d * 128:(d + 1) * 128],
                        xk_t[t][:, ki, :], start=(ki == 0), stop=(ki == KI - 1))
                cp_eng[d].tensor_copy(kTt[t][:, d, :], ps[:])
            units.append((8 * QT, ("proj", t), kunit))
        for tt in range(NT):
            def vunit(t=t, tt=tt):
                if tt == 0:
                    nc.gpsimd.tensor_copy(
                        vt[t][:, :, :, HD],
                        vones[:].rearrange("p (a b) -> p a b", a=NT))
                psv = psA.tile([128, DO], f32, tag="ps", name="psv")
                for ki in range(KI):
                    nc.tensor.matmul(
                        psv[:], xv_t[t][:, ki, tt * KT:(tt + 1) * KT],
                        wv_sb[:, ki, :], start=(ki == 0), stop=(ki == KI - 1))
                cp_eng[tt % 2].tensor_copy(
                    vt[t][:, tt, :, 0:HD],
                    psv[:].rearrange("p (h e) -> p h e", h=NH))
            units.append((8 * DO, ("proj", t), vunit))
        return units

    def oproj_units(t):
        units = []
        for mtt in range(NT):
            for n in range(D // QT):
                def ounit(t=t, mtt=mtt, n=n):
                    mt = t * NT + mtt
                    ps = psA.tile([128, QT], f32, tag="ps", name="pso2")
                    for kc in range(DO // 128):
                        nc.tensor.matmul(
                            ps[:], oTt[t][:, kc, mtt * KT:(mtt + 1) * KT],
                            wo_sb[:, kc, n * QT:(n + 1) * QT],
                            start=(kc == 0), stop=(kc == DO // 128 - 1))
                    ob = obuf.tile([128, QT], f32, tag="ob", name="ob")
                    cp_eng[(mtt + n) % 2].tensor_copy(ob[:], ps[:])
                    nc.sync.dma_start(
                        out_d[mt * 128:(mt + 1) * 128, n * QT:(n + 1) * QT],
                        ob[:])
                units.append((2 * QT, ("oproj", t), ounit))
        return units

    # ---- attention: scores -> exp (ACT) -> mask (Pool) -> PV, with the
    # filler stream keeping the PE dense; normalization is deferred into the
    # next block's stream so its serial chain hides behind attention work ----
    def attn_block(qt, reserve=0):
        LEAD = _opts["lead"]
        nkt = (qt + 1) * NT
        ouns = []
        for j in range(NH):
            poff = (j % 2) * HD
            d = j // 2
            qh = qTt[qt][poff:poff + HD, d, :]
            pso = psO.tile([HD + 1, QT], f32, tag="pso", name="pso")
            window = {}
            for step in range(nkt + LEAD):
                # PV first: its pt has been ready for LEAD chunks, so the PE
                # always has runnable work queued ahead of the (possibly
                # ACT-paced) score matmul behind it.
                if step >= LEAD:
                    kt = step - LEAD
                    co, w, pt = window.pop(kt)
                    nc.tensor.matmul(
                        pso[:, co:QT], vt[kt // NT][:, kt % NT, j, :],
                        pt[:, 0:w], start=(kt == 0), stop=(kt == nkt - 1))
                    drain(w, reserve)
                if step < nkt:
                    kt = step
                    r = kt - qt * NT
                    co = max(r, 0) * KT
                    w = QT - co
                    kh = kTt[kt // NT][poff:poff + HD, d,
                                       (kt % NT) * KT:(kt % NT + 1) * KT]
                    pss = psS.tile([128, QT], f32, tag="pss", name="pss")
                    nc.tensor.matmul(pss[:, 0:w], kh, qh[:, co:QT],
                                     start=True, stop=True)
                    pt = ptp.tile([128, QT], bf16, tag="pt", name="pt")
                    nc.scalar.activation(pt[:, 0:w], pss[:, 0:w], Exp,
                                         scale=0.125)
                    if r >= 0:
                        nc.gpsimd.tensor_mul(pt[:, 0:KT], pt[:, 0:KT],
                                             tri_sb[:])
                    window[kt] = (co, w, pt)
                    drain(w, reserve)
            # free the PSUM accumulator (DVE: keeps ACT's exp stream dense).
            # bufs=9: norm(qt) is emitted after attn(qt+1), so two blocks'
            # worth of oun tiles (8) are alive at once.
            oun = small.tile([HD + 1, QT], f32, tag="oun", name="oun", bufs=9)
            nc.vector.tensor_copy(oun[:], pso[:])
            if j == 0:
                rs = small.tile([128, QT], f32, tag="rs", name="rs")
                nc.gpsimd.memset(rs[:], 1.0)
            # park the rowsum at partition 32*j as soon as the head finishes
            # (DVE, directly behind the oun copy: Pool here would turn the
            # next block's in-order DVE queue into a cross-engine barrier)
            nc.vector.tensor_copy(rs[32 * j:32 * j + 1, :],
                                  oun[HD:HD + 1, :])
            ouns.append(oun)
        # rowsums -> 1/x on DVE (fast approx; sums are in [1, ~12000]).
        # Unused partitions hold 1.0 so the reciprocal stays finite; the
        # selector matmul zeroes them out anyway.
        rec_f = small.tile([128, QT], f32, tag="rec", name="rec")
        nc.vector.reciprocal_approx_fast(rec_f[:], rs[:])
        rec_r = small.tile([128, QT], f32r, tag="recr", name="recr")
        with nc.allow_low_precision(reason="f32r normalization scale"):
            nc.vector.tensor_copy(rec_r[:], rec_f[:])

        def norm(qt=qt, ouns=ouns, rec_r=rec_r):
            for dd in range(2):
                psb = psN.tile([128, QT], f32, tag="psb", name="psb")
                nc.tensor.matmul(psb[:], sel[:, dd * 128:(dd + 1) * 128],
                                 rec_r[:], start=True, stop=True)
                for pp in range(2):
                    nc.vector.tensor_mul(
                        oTt[qt][pp * HD:(pp + 1) * HD, dd, :],
                        ouns[2 * dd + pp][0:HD, :], psb[pp * HD:(pp + 1) * HD, :])
                drain(QT, 0)
        return norm

    # ---- schedule ----
    for u in proj_units(0):
        u[2]()
    fillers.extend(proj_units(1))
    fillers.extend(proj_units(2))
    fillers.extend(proj_units(3))

    norm_prev = attn_block(0)

    force_units(lambda tag: tag == ("proj", 1))
    norm_prev2 = attn_block(1)
    norm_prev()                       # norm0: its recip chain ran during attn1
    fillers.extend(oproj_units(0))
    norm_prev = norm_prev2

    force_units(lambda tag: tag == ("proj", 2))
    norm_prev2 = attn_block(2)
    norm_prev()
    fillers.extend(oproj_units(1))
    norm_prev = norm_prev2

    force_units(lambda tag: tag == ("proj", 3))
    norm_prev2 = attn_block(3, reserve=_opts["reserve"])
    norm_prev()
    fillers.extend(oproj_units(2))
    # drain everything except a small reserve to cover norm3's serial chain
    while len(fillers) > _opts["reserve"]:
        _cyc, _tag, fn = fillers.popleft()
        fn()
    norm_prev2()
    fillers.extend(oproj_units(3))
    force_units(lambda tag: True)


def _mask_tiles() -> np.ndarray:
    i = np.arange(128)[:, None]
    j = np.arange(KT)[None, :]
    return (j >= i).astype(ml_dtypes.bfloat16)


def _sel_mat() -> np.ndarray:
    p = np.arange(128)[:, None]
    c = np.arange(2 * HD * 2)[None, :]
    return (p == 32 * (c // HD)).astype(np.float32)


def make_in_maps(query, key, value, Wq, Wk, Wv, Wo):
    bf = ml_dtypes.bfloat16
    query = np.asarray(query, np.float32)
    key = np.asarray(key, np.float32)
    value = np.asarray(value, np.float32)
    Wq = np.asarray(Wq, np.float32)
    Wk = np.asarray(Wk, np.float32)
    Wv = np.asarray(Wv, np.float32)
    Wo = np.asarray(Wo, np.float32)
    cm = _mask_tiles()
    in_maps = []
    for c in range(NCORES):
        b, hg = divmod(c, NCORES // B)
        sl = slice(hg * DO, (hg + 1) * DO)
        in_maps.append({
            "xqT": np.ascontiguousarray(query[b].T).astype(bf),
            "xkT": np.ascontiguousarray(key[b].T).astype(bf),
            "xvT": np.ascontiguousarray(value[b].T).astype(bf),
            "wqT": np.ascontiguousarray(Wq[sl].T).astype(bf),
            "wkT": np.ascontiguousarray(Wk[sl].T).astype(bf),
            "wvT": np.ascontiguousarray(Wv[sl].T).astype(bf),
            "woT": np.ascontiguousarray(Wo[:, sl].T).astype(bf),
            "cmask": cm,
            "selr": _sel_mat(),
        })
    return in_maps


def kernel(query, key, value, freqs_complex_form, mask, Wq, Wk, Wv, Wo):
    if "nc" not in _cache:
        _cache["nc"] = _build()
    nc = _cache["nc"]
    in_maps = make_in_maps(query, key, value, Wq, Wk, Wv, Wo)
    res = run_bass_kernel_spmd(nc, in_maps, list(range(NCORES)))
    parts = [res.results[c]["out"] for c in range(NCORES)]
    npg = NCORES // B
    return np.stack(
        [np.sum(parts[b * npg:(b + 1) * npg], axis=0) for b in range(B)]
    ).astype(np.float32)


# revision 32
# speedup vs baseline: 1.1703x; 1.1607x over previous
"""Multi-head attention (B=2, S=2048, D=1024, H=16, causal) on 8 TRN2 NeuronCores.

Sharding: core c -> (batch b = c//4, head-group hg = c%4). Each core:
  - projects its batch's query/key/value against a 256-row slice of Wq/Wk/Wv
    (4 heads of 64 dims),
  - runs causal attention for those 4 heads (scores computed transposed,
    exp on ACT with fused 1/8 scale, row-sums via a ones-column in V),
  - multiplies by the matching 256-column slice of Wo -> partial [2048, 1024].
Host sums the 4 partials per batch (the tensor-parallel all-reduce) and stacks.

Performance structure: the TRN2 PE ramps to 2.4 GHz only after ~3us of
continuous busy time and drops to 1.2 GHz after any idle gap, and the per-chunk
exp stream on ACT is slower than the score+PV matmuls at full clock. So all
projection / output-projection matmul work is emitted as "filler" units
interleaved into the attention stream: whenever attention would wait on ACT,
the PE has independent proj/oproj work queued behind it. Everything runs in
bf16 (full-rate matmuls at any width, half the DMA/LDWEIGHTS traffic);
accumulation stays fp32 in PSUM. Normalization uses a fast DVE reciprocal and
a pair-packed selector matmul to broadcast the per-token scales.
"""

import sys

for _p in ("/opt/trn_rl_repo", "/root/.axon_site/_ro/trn_rl_repo"):
    if _p not in sys.path:
        sys.path.append(_p)

from collections import deque

import numpy as np
import ml_dtypes

import concourse.bacc as bacc
import concourse.tile as tile
import concourse.mybir as mybir
from concourse.bass import MemorySpace
from concourse.bass_utils import run_bass_kernel_spmd

f32 = mybir.dt.float32
f32r = mybir.dt.float32r
bf16 = mybir.dt.bfloat16
Exp = mybir.ActivationFunctionType.Exp

B, S, D, H = 2, 2048, 1024, 16
HD = 64            # head dim
NH = 4             # heads per core
DO = NH * HD       # 256 projection out-dims per core
NCORES = 8
KI = D // 128      # 8 contraction chunks for the projections
QT = 512           # query tile
NQT = S // QT      # 4
KT = 128           # key chunk (contraction tile for PV)
NKT = S // KT      # 16
NT = QT // KT      # 4 key chunks per query block

_cache: dict = {}

_opts = {"lead": 2, "ratio": 0.5, "reserve": 8}


def _build():
    nc = bacc.Bacc("TRN2", target_bir_lowering=False, debug=False,
                   num_devices=NCORES)

    xqT_d = nc.dram_tensor("xqT", [D, S], bf16, kind="ExternalInput").ap()
    xkT_d = nc.dram_tensor("xkT", [D, S], bf16, kind="ExternalInput").ap()
    xvT_d = nc.dram_tensor("xvT", [D, S], bf16, kind="ExternalInput").ap()
    wqT_d = nc.dram_tensor("wqT", [D, DO], bf16, kind="ExternalInput").ap()
    wkT_d = nc.dram_tensor("wkT", [D, DO], bf16, kind="ExternalInput").ap()
    wvT_d = nc.dram_tensor("wvT", [D, DO], bf16, kind="ExternalInput").ap()
    woT_d = nc.dram_tensor("woT", [DO, D], bf16, kind="ExternalInput").ap()
    cmask_d = nc.dram_tensor("cmask", [128, KT], bf16, kind="ExternalInput").ap()
    selr_d = nc.dram_tensor("selr", [128, 2 * HD * 2], f32r,
                            kind="ExternalInput").ap()
    out_d = nc.dram_tensor("out", [S, D], f32, kind="ExternalOutput").ap()

    with tile.TileContext(nc) as tc:
        with (
            tc.tile_pool(name="wpool", bufs=1) as wpool,
            tc.tile_pool(name="cpool", bufs=1) as cpool,
            tc.tile_pool(name="persist", bufs=1) as persist,
            tc.tile_pool(name="xin", bufs=12) as xin,
            tc.tile_pool(name="ptp", bufs=8) as ptp,
            tc.tile_pool(name="small", bufs=2) as small,
            tc.tile_pool(name="obuf", bufs=2) as obuf,
            tc.tile_pool(name="psS", bufs=3, space=MemorySpace.PSUM) as psS,
            tc.tile_pool(name="psA", bufs=2, space=MemorySpace.PSUM) as psA,
            tc.tile_pool(name="psO", bufs=2, space=MemorySpace.PSUM) as psO,
            tc.tile_pool(name="psN", bufs=1, space=MemorySpace.PSUM) as psN,
        ):
            _emit(nc, wpool, cpool, persist, xin, ptp, small, obuf,
                  psS, psA, psO, psN, xqT_d, xkT_d, xvT_d, wqT_d, wkT_d,
                  wvT_d, woT_d, cmask_d, selr_d, out_d)

    nc.compile()
    return nc


def _emit(nc, wpool, cpool, persist, xin, ptp, small, obuf, psS, psA, psO, psN,
          xqT_d, xkT_d, xvT_d, wqT_d, wkT_d, wvT_d, woT_d, cmask_d, selr_d,
          out_d):
    # ---- weights + constants; x DMAs issued up front (12 tiles coexist).
    # DMA order is the arrival order: tri (tiny, needed at first mask), then
    # weights/x interleaved so each proj unit's input lands just before the
    # PE reaches it; wo is only needed by oproj (~100us in) so it goes last.
    tri_sb = cpool.tile([128, KT], bf16, tag="tri")
    nc.sync.dma_start(tri_sb[:], cmask_d)
    xq_t, xk_t, xv_t = [], [], []

    def dma_x(lst, dram, t, name):
        ts = slice(t * QT, (t + 1) * QT)
        tl = xin.tile([128, KI, QT], bf16, tag="xin", name=f"{name}{t}")
        nc.sync.dma_start(tl[:], dram[:, ts].rearrange("(k p) n -> p k n",
                                                       p=128))
        lst.append(tl)

    wq_sb = wpool.tile([128, KI, DO], bf16, tag="wq")
    nc.sync.dma_start(wq_sb[:], wqT_d.rearrange("(k p) n -> p k n", p=128))
    dma_x(xq_t, xqT_d, 0, "xq")
    wk_sb = wpool.tile([128, KI, DO], bf16, tag="wk")
    nc.sync.dma_start(wk_sb[:], wkT_d.rearrange("(k p) n -> p k n", p=128))
    dma_x(xk_t, xkT_d, 0, "xk")
    dma_x(xq_t, xqT_d, 1, "xq")
    wv_sb = wpool.tile([128, KI, DO], bf16, tag="wv")
    nc.sync.dma_start(wv_sb[:], wvT_d.rearrange("(k p) n -> p k n", p=128))
    dma_x(xv_t, xvT_d, 0, "xv")
    dma_x(xk_t, xkT_d, 1, "xk")
    dma_x(xv_t, xvT_d, 1, "xv")
    wo_sb = wpool.tile([128, DO // 128, D], bf16, tag="wo")
    nc.sync.dma_start(wo_sb[:], woT_d.rearrange("(k p) n -> p k n", p=128))
    for t in range(2, NQT):
        dma_x(xq_t, xqT_d, t, "xq")
        dma_x(xk_t, xkT_d, t, "xk")
        dma_x(xv_t, xvT_d, t, "xv")

    # selector for the pair-packed reciprocal broadcast (host-built since
    # engine writes must start at 32-aligned partitions):
    # sel[p, c] = 1 iff p == 32 * (c // HD); rowsums are parked at
    # partitions {0,32,64,96} of the rs tile.
    sel = cpool.tile([128, 2 * HD * 2], f32r, tag="sel")
    nc.sync.dma_start(sel[:], selr_d)
    vones_f = cpool.tile([128, NT * NH], f32, tag="vones_f")
    nc.gpsimd.memset(vones_f[:], 1.0)
    vones = cpool.tile([128, NT * NH], bf16, tag="vones")
    nc.vector.tensor_copy(vones[:], vones_f[:])

    # ---- per-block persistent intermediates ----
    # qT/kT/oT blocks: [256, QT] as [128 parts, 2 chunks, QT]
    #   head j lives in chunk j//2, partitions (j%2)*64 ..+64
    qTt = [persist.tile([128, 2, QT], bf16, tag=f"qT{t}", name=f"qT{t}")
           for t in range(NQT)]
    kTt = [persist.tile([128, 2, QT], bf16, tag=f"kT{t}", name=f"kT{t}")
           for t in range(NQT)]
    oTt = [persist.tile([128, 2, QT], bf16, tag=f"oT{t}", name=f"oT{t}")
           for t in range(NQT)]
    # v blocks, natural layout + ones column: [tokk part, ktc, head, 65]
    vt = [persist.tile([128, NT, NH, HD + 1], bf16, tag=f"v{t}", name=f"v{t}")
          for t in range(NQT)]

    # ---- filler units: (cycles, tag, closure) drained into the PE stream ----
    fillers = deque()
    state = {"deficit": 0.0}
    RATIO = _opts["ratio"]

    def drain(cycles, reserve=0):
        state["deficit"] += cycles * RATIO
        while (len(fillers) > reserve
               and state["deficit"] >= fillers[0][0]):
            cyc, _tag, fn = fillers.popleft()
            fn()
            state["deficit"] -= cyc

    def force_units(pred):
        while fillers and pred(fillers[0][1]):
            _cyc, _tag, fn = fillers.popleft()
            fn()

    # Only DVE (and ACT) can drain PSUM on TRN2; ACT is saturated by the exp
    # stream, so every PSUM->SBUF copy goes to DVE and the SBUF-only work
    # (masks, rowsum parking) goes to Pool.
    cp_eng = [nc.vector, nc.vector]

    def proj_units(t):
        units = []
        for d in range(2):
            def qunit(t=t, d=d):
                ps = psA.tile([128, QT], f32, tag="ps", name="psq")
                for ki in range(KI):
                    nc.tensor.matmul(
                        ps[:], wq_sb[:, ki, d * 128:(d + 1) * 128],
                        xq_t[t][:, ki, :], start=(ki == 0), stop=(ki == KI - 1))
                cp_eng[d].tensor_copy(qTt[t][:, d, :], ps[:])
            units.append((8 * QT, ("proj", t), qunit))
        for d in range(2):
            def kunit(t=t, d=d):
                ps = psA.tile([128, QT], f32, tag="ps", name="psk")
                for ki in range(KI):
                    nc.tensor.matmul(
                        ps[:], wk_sb[:, ki, d * 128:(d + 1) * 128],
                        xk_t[t][:, ki, :], start=(ki == 0), stop=(ki == KI - 1))
                cp_eng[d].tensor_copy(kTt[t][:, d, :], ps[:])
            units.append((8 * QT, ("proj", t), kunit))
        for tt in range(NT):
            def vunit(t=t, tt=tt):
                if tt == 0:
                    nc.gpsimd.tensor_copy(
                        vt[t][:, :, :, HD],
                        vones[:].rearrange("p (a b) -> p a b", a=NT))
                psv = psA.tile([128, DO], f32, tag="ps", name="psv")
                for ki in range(KI):
                    nc.tensor.matmul(
                        psv[:], xv_t[t][:, ki, tt * KT:(tt + 1) * KT],
                        wv_sb[:, ki, :], start=(ki == 0), stop=(ki == KI - 1))
                cp_eng[tt % 2].tensor_copy(
                    vt[t][:, tt, :, 0:HD],
                    psv[:].rearrange("p (h e) -> p h e", h=NH))
            units.append((8 * DO, ("proj", t), vunit))
        return units

    def oproj_units(t):
        units = []
        for mtt in range(NT):
            for n in range(D // QT):
                def ounit(t=t, mtt=mtt, n=n):
                    mt = t * NT + mtt
                    ps = psA.tile([128, QT], f32, tag="ps", name="pso2")
                    for kc in range(DO // 128):
                        nc.tensor.matmul(
                            ps[:], oTt[t][:, kc, mtt * KT:(mtt + 1) * KT],
                            wo_sb[:, kc, n * QT:(n + 1) * QT],
                            start=(kc == 0), stop=(kc == DO // 128 - 1))
                    ob = obuf.tile([128, QT], f32, tag="ob", name="ob")
                    cp_eng[(mtt + n) % 2].tensor_copy(ob[:], ps[:])
                    nc.sync.dma_start(
                        out_d[mt * 128:(mt + 1) * 128, n * QT:(n + 1) * QT],
                        ob[:])
                units.append((2 * QT, ("oproj", t), ounit))
        return units

    # ---- attention: scores -> exp (ACT) -> mask (Pool) -> PV, with the
    # filler stream keeping the PE dense; normalization is deferred into the
    # next block's stream so its serial chain hides behind attention work ----
    def attn_block(qt, reserve=0):
        LEAD = _opts["lead"]
        nkt = (qt + 1) * NT
        ouns = []
        for j in range(NH):
            poff = (j % 2) * HD
            d = j // 2
            qh = qTt[qt][poff:poff + HD, d, :]
            pso = psO.tile([HD + 1, QT], f32, tag="pso", name="pso")
            window = {}
            for step in range(nkt + LEAD):
                if step < nkt:
                    kt = step
                    r = kt - qt * NT
                    co = max(r, 0) * KT
                    w = QT - co
                    kh = kTt[kt // NT][poff:poff + HD, d,
                                       (kt % NT) * KT:(kt % NT + 1) * KT]
                    pss = psS.tile([128, QT], f32, tag="pss", name="pss")
                    nc.tensor.matmul(pss[:, 0:w], kh, qh[:, co:QT],
                                     start=True, stop=True)
                    pt = ptp.tile([128, QT], bf16, tag="pt", name="pt")
                    nc.scalar.activation(pt[:, 0:w], pss[:, 0:w], Exp,
                                         scale=0.125)
                    if r >= 0:
                        nc.gpsimd.tensor_mul(pt[:, 0:KT], pt[:, 0:KT],
                                             tri_sb[:])
                    window[kt] = (co, w, pt)
                    drain(w, reserve)
                if step >= LEAD:
                    kt = step - LEAD
                    co, w, pt = window.pop(kt)
                    nc.tensor.matmul(
                        pso[:, co:QT], vt[kt // NT][:, kt % NT, j, :],
                        pt[:, 0:w], start=(kt == 0), stop=(kt == nkt - 1))
                    drain(w, reserve)
            # free the PSUM accumulator (DVE: keeps ACT's exp stream dense).
            # bufs=9: norm(qt) is emitted after attn(qt+1), so two blocks'
            # worth of oun tiles (8) are alive at once.
            oun = small.tile([HD + 1, QT], f32, tag="oun", name="oun", bufs=9)
            nc.vector.tensor_copy(oun[:], pso[:])
            if j == 0:
                rs = small.tile([128, QT], f32, tag="rs", name="rs")
                nc.gpsimd.memset(rs[:], 1.0)
            # park the rowsum at partition 32*j as soon as the head finishes
            # (DVE, directly behind the oun copy: Pool here would turn the
            # next block's in-order DVE queue into a cross-engine barrier)
            nc.vector.tensor_copy(rs[32 * j:32 * j + 1, :],
                                  oun[HD:HD + 1, :])
            ouns.append(oun)
        # rowsums -> 1/x on DVE (fast approx; sums are in [1, ~12000]).
        # Unused partitions hold 1.0 so the reciprocal stays finite; the
        # selector matmul zeroes them out anyway.
        rec_f = small.tile([128, QT], f32, tag="rec", name="rec")
        nc.vector.reciprocal_approx_fast(rec_f[:], rs[:])
        rec_r = small.tile([128, QT], f32r, tag="recr", name="recr")
        with nc.allow_low_precision(reason="f32r normalization scale"):
            nc.vector.tensor_copy(rec_r[:], rec_f[:])

        def norm(qt=qt, ouns=ouns, rec_r=rec_r):
            for dd in range(2):
                psb = psN.tile([128, QT], f32, tag="psb", name="psb")
                nc.tensor.matmul(psb[:], sel[:, dd * 128:(dd + 1) * 128],
                                 rec_r[:], start=True, stop=True)
                for pp in range(2):
                    nc.vector.tensor_mul(
                        oTt[qt][pp * HD:(pp + 1) * HD, dd, :],
                        ouns[2 * dd + pp][0:HD, :], psb[pp * HD:(pp + 1) * HD, :])
                drain(QT, 0)
        return norm

    # ---- schedule ----
    for u in proj_units(0):
        u[2]()
    fillers.extend(proj_units(1))
    fillers.extend(proj_units(2))
    fillers.extend(proj_units(3))

    norm_prev = attn_block(0)

    force_units(lambda tag: tag == ("proj", 1))
    norm_prev2 = attn_block(1)
    norm_prev()                       # norm0: its recip chain ran during attn1
    fillers.extend(oproj_units(0))
    norm_prev = norm_prev2

    force_units(lambda tag: tag == ("proj", 2))
    norm_prev2 = attn_block(2)
    norm_prev()
    fillers.extend(oproj_units(1))
    norm_prev = norm_prev2

    force_units(lambda tag: tag == ("proj", 3))
    norm_prev2 = attn_block(3, reserve=_opts["reserve"])
    norm_prev()
    fillers.extend(oproj_units(2))
    # drain everything except a small reserve to cover norm3's serial chain
    while len(fillers) > _opts["reserve"]:
        _cyc, _tag, fn = fillers.popleft()
        fn()
    norm_prev2()
    fillers.extend(oproj_units(3))
    force_units(lambda tag: True)


def _mask_tiles() -> np.ndarray:
    i = np.arange(128)[:, None]
    j = np.arange(KT)[None, :]
    return (j >= i).astype(ml_dtypes.bfloat16)


def _sel_mat() -> np.ndarray:
    p = np.arange(128)[:, None]
    c = np.arange(2 * HD * 2)[None, :]
    return (p == 32 * (c // HD)).astype(np.float32)


def make_in_maps(query, key, value, Wq, Wk, Wv, Wo):
    bf = ml_dtypes.bfloat16
    query = np.asarray(query, np.float32)
    key = np.asarray(key, np.float32)
    value = np.asarray(value, np.float32)
    Wq = np.asarray(Wq, np.float32)
    Wk = np.asarray(Wk, np.float32)
    Wv = np.asarray(Wv, np.float32)
    Wo = np.asarray(Wo, np.float32)
    cm = _mask_tiles()
    in_maps = []
    for c in range(NCORES):
        b, hg = divmod(c, NCORES // B)
        sl = slice(hg * DO, (hg + 1) * DO)
        in_maps.append({
            "xqT": np.ascontiguousarray(query[b].T).astype(bf),
            "xkT": np.ascontiguousarray(key[b].T).astype(bf),
            "xvT": np.ascontiguousarray(value[b].T).astype(bf),
            "wqT": np.ascontiguousarray(Wq[sl].T).astype(bf),
            "wkT": np.ascontiguousarray(Wk[sl].T).astype(bf),
            "wvT": np.ascontiguousarray(Wv[sl].T).astype(bf),
            "woT": np.ascontiguousarray(Wo[:, sl].T).astype(bf),
            "cmask": cm,
            "selr": _sel_mat(),
        })
    return in_maps


def kernel(query, key, value, freqs_complex_form, mask, Wq, Wk, Wv, Wo):
    if "nc" not in _cache:
        _cache["nc"] = _build()
    nc = _cache["nc"]
    in_maps = make_in_maps(query, key, value, Wq, Wk, Wv, Wo)
    res = run_bass_kernel_spmd(nc, in_maps, list(range(NCORES)))
    parts = [res.results[c]["out"] for c in range(NCORES)]
    npg = NCORES // B
    return np.stack(
        [np.sum(parts[b * npg:(b + 1) * npg], axis=0) for b in range(B)]
    ).astype(np.float32)
